# revision 2
# baseline (speedup 1.0000x reference)
"""CRF NLL kernel for Trainium2 (8 NeuronCores), time-sharded forward algorithm.

Math: NLL[b] = logZ[b] - gold_score[b].

logZ is computed with the scaled forward algorithm in exp space:
  q_t = (expT'^T q_{t-1}) * exp(e_t)   with expT' = exp(T - MU)
so each scan step is a (256x256) @ (256xB) matmul plus an elementwise
multiply.  The per-step constant rescale e^{-MU} keeps magnitudes in fp
range (validated on the dataset: cumulative drift stays within [-30, 1]).

Time sharding: the 1024 steps are split into 8 blocks of 128 (one per
core).  Each core warm-starts W=32 steps early from a uniform state: the
positive-matrix scan forgets its initialization at ~0.1/step, so after 32
steps the normalized state direction matches the true trajectory to
~1e-16.  Each core reports log||state|| after warm-up (lw), after its
block (le), and the EOS-weighted log-norm (fin).  Scale invariance gives
the exact block contribution delta_c = le_c - lw_c, and
  logZ = sum_c delta_c + 1024*MU + (fin_7 - le_7).
Core 0 has no "earlier" steps; its warm-up window ends with a BOS mask
slice (log-space one-hot) that forces the state onto the exact t=0
initial condition, making its block exact as well.

The gold score (gather of 2*S elements per sequence) is evaluated on the
host: it is 0.002% of the FLOPs and none of the memory traffic.
"""

import numpy as np

B, S, L = 128, 1024, 256
NCORES = 8
W = 16                 # warm-up steps per core
NT = W + S // NCORES   # 160 slices per core
TCH = 16               # timesteps per DMA chunk
NCHUNK = NT // TCH
MU = 6.7
BOS, EOS = 0, 1

_CACHE = {}


def _build_nc(reps=1, W_=None, variant="jc", tch=None, bench_small=False):
    import concourse.bacc as bacc
    import concourse.tile as tile
    import concourse.mybir as mybir

    Wl = W if W_ is None else W_
    TCHl = TCH if tch is None else tch
    NTl = Wl + S // NCORES
    nchunk = NTl // TCHl
    assert nchunk * TCHl == NTl

    f32 = mybir.dt.float32
    bf16 = mybir.dt.bfloat16
    Act = mybir.ActivationFunctionType

    nc = bacc.Bacc(
        "TRN2", target_bir_lowering=False, debug=False, num_devices=NCORES
    )
    emis = nc.dram_tensor("emis", [TCHl if bench_small else NTl, 128, 256], f32, kind="ExternalInput")
    trans = nc.dram_tensor("trans", [2, 128, 256], f32, kind="ExternalInput")
    teos = nc.dram_tensor("teos", [2, 128, 1], f32, kind="ExternalInput")
    outv = nc.dram_tensor("outv", [1, 384], f32, kind="ExternalOutput")

    with tile.TileContext(nc) as tc:
        with (
            tc.tile_pool(name="const", bufs=1) as cpool,
            tc.tile_pool(name="stage", bufs=2) as spool,
            tc.tile_pool(name="emchunk", bufs=2) as epool,
            tc.tile_pool(name="expchunk", bufs=2) as xpool,
            tc.tile_pool(name="qa", bufs=4) as qapool,
            tc.tile_pool(name="qb", bufs=4) as qbpool,
            tc.tile_pool(name="ps", bufs=3, space="PSUM") as ppool,
            tc.tile_pool(name="psn", bufs=2, space="PSUM") as npool,
            tc.tile_pool(name="outs", bufs=1) as opool,
        ):
            negmu = cpool.tile([128, 1], f32, tag="negmu")
            nc.gpsimd.memset(negmu[:], -MU)
            zbias = cpool.tile([128, 1], f32, tag="zbias")
            nc.gpsimd.memset(zbias[:], 0.0)
            zbias1 = cpool.tile([1, 1], f32, tag="zbias1")
            nc.gpsimd.memset(zbias1[:], 0.0)
            # transition weights: expT' = exp(T - MU), bf16, as 2 K-chunk tiles
            wT = []
            for ic in range(2):
                st = spool.tile([128, 256], f32, tag="stage", name=f"st{ic}")
                nc.sync.dma_start(st[:], trans[ic])
                w = cpool.tile([128, 256], bf16, tag=f"wT{ic}", name=f"wT{ic}")
                nc.scalar.activation(w[:], st[:], Act.Exp, bias=negmu[:])
                wT.append(w)
            # EOS column weights: exp(T[:, EOS]) (no MU)
            wTe = []
            for ic in range(2):
                st = spool.tile([128, 1], f32, tag="stagee", name=f"ste{ic}")
                nc.sync.dma_start(st[:], teos[ic])
                w = cpool.tile([128, 1], bf16, tag=f"wTe{ic}", name=f"wTe{ic}")
                nc.scalar.activation(w[:], st[:], Act.Exp, bias=zbias[:])
                wTe.append(w)
            ones_col = cpool.tile([128, 1], bf16, tag="ones")
            nc.gpsimd.memset(ones_col[:], 1.0)

            out_sb = opool.tile([1, 384], f32, tag="outsb")

            for rep in range(reps):
                # state tiles, one per label-chunk: qj[jc][p, b] = q[jc*128+p, b]
                qj = []
                for jc in range(2):
                    q0 = (qapool if jc == 0 else qbpool).tile(
                        [128, 128], bf16, tag=f"q{jc}", name=f"qinit{jc}_{rep}"
                    )
                    nc.gpsimd.memset(q0[:], 1.0 / L)
                    qj.append(q0)

                for ch in range(nchunk):
                    et = epool.tile(
                        [128, TCHl * 256], f32, tag="et", name=f"et_{rep}_{ch}"
                    )
                    srcsl = (
                        emis[0:TCHl] if bench_small
                        else emis[ch * TCHl : (ch + 1) * TCHl]
                    )
                    nc.sync.dma_start(
                        et.rearrange("p (t x) -> p t x", t=TCHl),
                        srcsl.rearrange("t p x -> p t x"),
                    )
                    xt = xpool.tile(
                        [128, TCHl * 256], f32, tag="xt", name=f"xt_{rep}_{ch}"
                    )
                    nc.scalar.activation(xt[:], et[:], Act.Exp, bias=zbias[:])

                    for s in range(TCHl):
                        t = ch * TCHl + s
                        pts = [
                            ppool.tile(
                                [128, 128], f32, tag=f"pt{jc}",
                                name=f"pt{jc}_{rep}_{t}",
                            )
                            for jc in range(2)
                        ]
                        qn = [
                            (qapool if jc == 0 else qbpool).tile(
                                [128, 128], bf16, tag=f"q{jc}",
                                name=f"q{jc}_{rep}_{t}",
                            )
                            for jc in range(2)
                        ]
                        # out[jc*128+p, b] = sum_ic wT[ic][:, jc].T @ qj[ic]
                        for ic in range(2):
                            for jc in range(2):
                                nc.tensor.matmul(
                                    pts[jc][:],
                                    wT[ic][:, jc * 128 : (jc + 1) * 128],
                                    qj[ic][:],
                                    start=(ic == 0),
                                    stop=(ic == 1),
                                )
                            if ic == 1:
                                for jc in range(2):
                                    nc.vector.tensor_mul(
                                        qn[jc][:],
                                        pts[jc][:],
                                        xt[:, s * 256 + jc * 128 : s * 256 + (jc + 1) * 128],
                                    )
                        qj = qn

                        if t == Wl - 1 or t == NTl - 1:
                            nt = npool.tile(
                                [1, 128], f32, tag="nt", name=f"nt_{rep}_{t}"
                            )
                            for ic in range(2):
                                nc.tensor.matmul(
                                    nt[:],
                                    ones_col[:],
                                    qj[ic][:],
                                    start=(ic == 0),
                                    stop=(ic == 1),
                                )
                            row = 0 if t == Wl - 1 else 1
                            nc.scalar.activation(
                                out_sb[:, row * 128 : (row + 1) * 128],
                                nt[:],
                                Act.Ln,
                                bias=zbias1[:],
                            )
                        if t == NTl - 1:
                            nt = npool.tile(
                                [1, 128], f32, tag="nt", name=f"ntf_{rep}_{t}"
                            )
                            for ic in range(2):
                                nc.tensor.matmul(
                                    nt[:],
                                    wTe[ic][:],
                                    qj[ic][:],
                                    start=(ic == 0),
                                    stop=(ic == 1),
                                )
                            nc.scalar.activation(
                                out_sb[:, 256:384], nt[:], Act.Ln, bias=zbias1[:]
                            )

            nc.sync.dma_start(outv[:], out_sb[:])

    nc.compile()
    return nc


def _pack_emis(em_block):
    """(B=128, T, L=256) -> (T, 128, 256) with [t, p, c*128+b] = em[b, t, c*128+p]."""
    T = em_block.shape[1]
    arr = np.ascontiguousarray(em_block.transpose(1, 2, 0))  # (T, L, B)
    arr = arr.reshape(T, 2, 128, 128).transpose(0, 2, 1, 3)  # (T, 128, 2, 128)
    return np.ascontiguousarray(arr.reshape(T, 128, 256), dtype=np.float32)


def kernel(emissions, tags, mask, transitions):
    from concourse.bass_utils import run_bass_kernel_spmd

    emissions = np.asarray(emissions, dtype=np.float32)
    tags_i = np.asarray(tags).astype(np.int64)
    transitions = np.asarray(transitions, dtype=np.float32)

    if "nc" not in _CACHE:
        _CACHE["nc"] = _build_nc()
    nc = _CACHE["nc"]

    trans_in = np.ascontiguousarray(transitions.reshape(2, 128, 256))
    teos_in = np.ascontiguousarray(
        transitions[:, EOS].reshape(2, 128, 1)
    )

    blk = S // NCORES
    in_maps = []
    for c in range(NCORES):
        t0 = c * blk
        if c == 0:
            em = np.empty((NT, 128, 256), dtype=np.float32)
            em[: W - 1] = _pack_emis(emissions[:, : W - 1, :])
            # BOS mask slice in log space: 0 at l==BOS, -1e9 elsewhere
            m = np.full((128, 256), -1e9, dtype=np.float32)
            m[BOS % 128, (BOS // 128) * 128 : (BOS // 128) * 128 + 128] = 0.0
            em[W - 1] = m
            em[W:] = _pack_emis(emissions[:, t0 : t0 + blk, :])
        else:
            em = _pack_emis(emissions[:, t0 - W : t0 + blk, :])
        in_maps.append({"emis": em, "trans": trans_in, "teos": teos_in})

    res = run_bass_kernel_spmd(nc, in_maps, list(range(NCORES)))
    _CACHE["last_res"] = res
    outs = np.stack([np.asarray(r["outv"]).reshape(3, 128) for r in res.results])

    lw = outs[:, 0, :].astype(np.float64)
    le = outs[:, 1, :].astype(np.float64)
    fin = outs[:, 2, :].astype(np.float64)
    logZ = (le - lw).sum(axis=0) + S * MU + (fin[-1] - le[-1])

    # gold path score on host (tiny: 2*S gathers per sequence)
    em64 = emissions.astype(np.float64)
    T64 = transitions.astype(np.float64)
    e_all = np.take_along_axis(em64, tags_i[..., None], axis=2).squeeze(-1)
    t_all = T64[tags_i[:, :-1], tags_i[:, 1:]]
    scores = (
        T64[BOS, tags_i[:, 0]]
        + e_all[:, 0]
        + (e_all[:, 1:] + t_all).sum(axis=1)
        + T64[tags_i[:, -1], EOS]
    )
    return (logZ - scores).astype(np.float32)



# revision 4
# speedup vs baseline: 1.8104x; 1.8104x over previous
"""CRF NLL kernel for Trainium2 (8 NeuronCores), paired-chain time-sharded
forward algorithm.

Math: NLL[b] = logZ[b] - gold_score[b].

logZ uses the scaled forward algorithm in exp space:
  q_t = (expT'^T q_{t-1}) * exp(e_t)   with expT' = exp(T - MU)
so each step is a (256x256) @ (256xB) matmul plus an elementwise multiply.
The constant per-step rescale e^{-MU} keeps magnitudes in fp range.

Sharding: the 1024 steps are split into 32 shards of 32 steps; each core
runs 4 shards ("chains").  Chains warm-start W=4 steps early from a
uniform state (the positive-matrix scan forgets its init fast; validated
max rel err ~1.5e-7 in f64 at W=4).  Shard 0 instead uses a BOS one-hot
mask slice, making it exact.  Scale invariance gives each shard's
contribution via log-norms taken after warmup (lw) and at the end (le):
  logZ = sum_c (le_c - lw_c) + S*MU + (fin_31 - le_31)
where fin is the EOS-weighted log-norm of the last shard.

On-chip layout: chains are PAIRED so each matmul's moving operand is
[128, 256] (two chains' batches side by side), which hides the 107 ns
LDWEIGHTS behind the 256-column stream.  The two pairs per core are
interleaved step-by-step so the DVE multiply of one pair overlaps the
PE matmuls of the other.  Per pair-step: 4 matmuls accumulate a
[128, 512] PSUM tile (one bank), then ONE fused DVE multiply
(psum * exp(e)) -> bf16 state tile.

The gold path score is evaluated on the host (0.002% of FLOPs, none of
the memory traffic).
"""

import numpy as np

B, S, L = 128, 1024, 256
NCORES = 8
NCHAIN = 4              # chains (shards) per core
NSH = NCORES * NCHAIN   # 32 shards
BLK = S // NSH          # 32 steps per shard
W = 4                   # warm-up steps per chain
NPS = 2 * (BLK + W)     # 72 pair-steps per core
NSL = 2 * NPS           # 144 slices (chain-steps) per core
TCH = 8                 # pair-steps per DMA chunk
NCHUNK = NPS // TCH     # 9
MU = 6.7
BOS, EOS = 0, 1

_CACHE = {}


def _build_nc():
    import concourse.bacc as bacc
    import concourse.tile as tile
    import concourse.mybir as mybir

    f32 = mybir.dt.float32
    bf16 = mybir.dt.bfloat16
    Act = mybir.ActivationFunctionType

    nc = bacc.Bacc(
        "TRN2", target_bir_lowering=False, debug=False, num_devices=NCORES
    )
    # p-major packed emissions: [p, pair_step*512 + jc*256 + half*128 + b]
    emis = nc.dram_tensor("emis", [128, NPS * 512], f32, kind="ExternalInput")
    trans = nc.dram_tensor("trans", [2, 128, 256], f32, kind="ExternalInput")
    teos = nc.dram_tensor("teos", [2, 128, 1], f32, kind="ExternalInput")
    outv = nc.dram_tensor("outv", [1, 1152], f32, kind="ExternalOutput")

    with tile.TileContext(nc) as tc:
        with (
            tc.tile_pool(name="const", bufs=1) as cpool,
            tc.tile_pool(name="stage", bufs=2) as spool,
            tc.tile_pool(name="emchunk", bufs=2) as epool,
            tc.tile_pool(name="expchunk", bufs=2) as xpool,
            tc.tile_pool(name="qa", bufs=3) as qapool,
            tc.tile_pool(name="qb", bufs=3) as qbpool,
            tc.tile_pool(name="ps", bufs=4, space="PSUM") as ppool,
            tc.tile_pool(name="psn", bufs=2, space="PSUM") as npool,
            tc.tile_pool(name="outs", bufs=1) as opool,
        ):
            negmu = cpool.tile([128, 1], f32, tag="negmu")
            nc.gpsimd.memset(negmu[:], -MU)
            zbias = cpool.tile([128, 1], f32, tag="zbias")
            nc.gpsimd.memset(zbias[:], 0.0)
            zbias1 = cpool.tile([1, 1], f32, tag="zbias1")
            nc.gpsimd.memset(zbias1[:], 0.0)
            # transition weights: expT' = exp(T - MU), bf16, 2 K-chunk tiles
            wT = []
            for ic in range(2):
                st = spool.tile([128, 256], f32, tag="stage", name=f"st{ic}")
                nc.sync.dma_start(st[:], trans[ic])
                w = cpool.tile([128, 256], bf16, tag=f"wT{ic}", name=f"wT{ic}")
                nc.scalar.activation(w[:], st[:], Act.Exp, bias=negmu[:])
                wT.append(w)
            # EOS column weights: exp(T[:, EOS]) (no MU)
            wTe = []
            for ic in range(2):
                st = spool.tile([128, 1], f32, tag="stagee", name=f"ste{ic}")
                nc.sync.dma_start(st[:], teos[ic])
                w = cpool.tile([128, 1], bf16, tag=f"wTe{ic}", name=f"wTe{ic}")
                nc.scalar.activation(w[:], st[:], Act.Exp, bias=zbias[:])
                wTe.append(w)
            ones_col = cpool.tile([128, 1], bf16, tag="ones")
            nc.gpsimd.memset(ones_col[:], 1.0)

            out_sb = opool.tile([1, 1152], f32, tag="outsb")

            # state per pair: [128, 512] bf16; cols = ic*256 + half*128 + b
            qcur = []
            for pi in range(2):
                q0 = (qapool if pi == 0 else qbpool).tile(
                    [128, 512], bf16, tag=f"q{pi}", name=f"qinit{pi}"
                )
                nc.gpsimd.memset(q0[:], 1.0 / L)
                qcur.append(q0)

            for ch in range(NCHUNK):
                et = epool.tile([128, TCH * 512], f32, tag="et", name=f"et_{ch}")
                nc.sync.dma_start(
                    et[:], emis[:, ch * TCH * 512 : (ch + 1) * TCH * 512]
                )
                xt = xpool.tile([128, TCH * 512], f32, tag="xt", name=f"xt_{ch}")
                nc.scalar.activation(xt[:], et[:], Act.Exp, bias=zbias[:])

                for s in range(TCH):
                    g = ch * TCH + s      # global pair-step 0..NPS-1
                    pi = g & 1            # which pair
                    step = g >> 1         # 0..BLK+W-1 within the pair
                    q = qcur[pi]
                    pts = ppool.tile(
                        [128, 512], f32, tag="pt", name=f"pt_{g}"
                    )
                    # psum cols = jc*256 + half*128 + b
                    for jc in range(2):
                        for ic in range(2):
                            nc.tensor.matmul(
                                pts[:, jc * 256 : (jc + 1) * 256],
                                wT[ic][:, jc * 128 : (jc + 1) * 128],
                                q[:, ic * 256 : (ic + 1) * 256],
                                start=(ic == 0),
                                stop=(ic == 1),
                            )
                    qn = (qapool if pi == 0 else qbpool).tile(
                        [128, 512], bf16, tag=f"q{pi}", name=f"q{pi}_{g}"
                    )
                    nc.vector.tensor_mul(
                        qn[:], pts[:], xt[:, s * 512 : (s + 1) * 512]
                    )
                    qcur[pi] = qn

                    if step == W - 1 or step == W + BLK - 1:
                        # log-norms per half: rows 2k (lw) / 2k+1 (le),
                        # chain local k = 2*pi + half
                        for half in range(2):
                            k = 2 * pi + half
                            row = 2 * k + (0 if step == W - 1 else 1)
                            nt = npool.tile(
                                [1, 128], f32, tag="nt", name=f"nt_{g}_{half}"
                            )
                            for ic in range(2):
                                nc.tensor.matmul(
                                    nt[:],
                                    ones_col[:],
                                    qn[:, ic * 256 + half * 128 : ic * 256 + half * 128 + 128],
                                    start=(ic == 0),
                                    stop=(ic == 1),
                                )
                            nc.scalar.activation(
                                out_sb[:, row * 128 : (row + 1) * 128],
                                nt[:],
                                Act.Ln,
                                bias=zbias1[:],
                            )
                        if pi == 1 and step == W + BLK - 1:
                            # EOS-weighted log-norm of the core's last chain
                            ntf = npool.tile([1, 128], f32, tag="nt", name="ntf")
                            for ic in range(2):
                                nc.tensor.matmul(
                                    ntf[:],
                                    wTe[ic][:],
                                    qn[:, ic * 256 + 128 : ic * 256 + 256],
                                    start=(ic == 0),
                                    stop=(ic == 1),
                                )
                            nc.scalar.activation(
                                out_sb[:, 1024:1152], ntf[:], Act.Ln,
                                bias=zbias1[:],
                            )

            nc.sync.dma_start(outv[:], out_sb[:])

    nc.compile()
    return nc


def _pack_all(emissions):
    """Pack (B,S,L) emissions into per-core p-major arrays.

    out[c][p, ((2*step+pi)*2 + jc)*256 + half*128 + b]
        = emissions[b, t(shard=4c+2pi+half, step), jc*128+p]
    """
    el4 = np.ascontiguousarray(emissions.transpose(2, 1, 0)).reshape(
        2, 128, S, B
    )  # [jc, p, t, b]
    nst = BLK + W
    tmap = np.empty((NCORES, 2, 2, nst), dtype=np.int64)
    for sh in range(NSH):
        t0 = sh * BLK
        c, r = divmod(sh, NCHAIN)
        pi, half = divmod(r, 2)
        if sh == 0:
            tmap[c, pi, half, : W - 1] = np.arange(W - 1)  # junk steps
            tmap[c, pi, half, W - 1] = 0                   # BOS mask slot
            tmap[c, pi, half, W:] = np.arange(BLK)
        else:
            tmap[c, pi, half] = np.arange(t0 - W, t0 + BLK)
    g = el4[:, :, tmap, :]  # [jc, p, c, pi, half, st, b]
    g = g.transpose(2, 1, 5, 3, 0, 4, 6)  # [c, p, st, pi, jc, half, b]
    ems = np.ascontiguousarray(g.reshape(NCORES, 128, NPS * 512))
    # BOS mask slice for shard 0 (core 0, pair 0, half 0, step W-1):
    # pair-step 2*(W-1)+0, cols jc*256 + 0*128 + b
    base = (2 * (W - 1)) * 512
    for jc in range(2):
        ems[0, :, base + jc * 256 : base + jc * 256 + 128] = -1e9
    ems[0, BOS, base : base + 128] = 0.0
    return ems


def kernel(emissions, tags, mask, transitions):
    from concourse.bass_utils import run_bass_kernel_spmd

    emissions = np.asarray(emissions, dtype=np.float32)
    tags_i = np.asarray(tags).astype(np.int64)
    transitions = np.asarray(transitions, dtype=np.float32)

    if "nc" not in _CACHE:
        _CACHE["nc"] = _build_nc()
    nc = _CACHE["nc"]

    trans_in = np.ascontiguousarray(transitions.reshape(2, 128, 256))
    teos_in = np.ascontiguousarray(transitions[:, EOS].reshape(2, 128, 1))

    ems = _pack_all(emissions)
    in_maps = [
        {"emis": ems[c], "trans": trans_in, "teos": teos_in}
        for c in range(NCORES)
    ]

    res = run_bass_kernel_spmd(nc, in_maps, list(range(NCORES)))
    _CACHE["last_res"] = res
    outs = np.stack(
        [np.asarray(r["outv"]).reshape(9, 128) for r in res.results]
    )  # [core, row, b]

    lw = outs[:, 0:8:2, :].astype(np.float64)  # [core, k, b]
    le = outs[:, 1:8:2, :].astype(np.float64)
    fin = outs[-1, 8, :].astype(np.float64)
    logZ = (le - lw).sum(axis=(0, 1)) + S * MU + (fin - le[-1, -1])

    # gold path score on host (tiny: 2*S gathers per sequence)
    em64 = emissions.astype(np.float64)
    T64 = transitions.astype(np.float64)
    e_all = np.take_along_axis(em64, tags_i[..., None], axis=2).squeeze(-1)
    t_all = T64[tags_i[:, :-1], tags_i[:, 1:]]
    scores = (
        T64[BOS, tags_i[:, 0]]
        + e_all[:, 0]
        + (e_all[:, 1:] + t_all).sum(axis=1)
        + T64[tags_i[:, -1], EOS]
    )
    return (logZ - scores).astype(np.float32)


# revision 5
# speedup vs baseline: 2.1345x; 1.1790x over previous
"""CRF NLL kernel for Trainium2 (8 NeuronCores), paired-chain time-sharded
forward algorithm.

Math: NLL[b] = logZ[b] - gold_score[b].

logZ uses the scaled forward algorithm in exp space:
  q_t = (expT'^T q_{t-1}) * x_t,   expT' = exp(T - MU),  x_t = exp(e_t)
so each step is a (256x256) @ (256xB) matmul plus an elementwise multiply.
The constant per-step rescale e^{-MU} keeps magnitudes in fp range.

Sharding: 1024 steps -> 64 shards of 16 steps; each core runs 8 shards
("chains").  Chains warm-start W=4 steps early from a uniform state (the
positive-matrix scan forgets its init fast; validated ~2e-6 rel err with
bf16 x).  Shard 0 instead gets an exact BOS one-hot mask slice.  Scale
invariance gives each shard's contribution from log-norms after warmup
(lw) and at the end (le):
  logZ = sum_c (le_c - lw_c) + S*MU + (fin_63 - le_63)

On-chip layout: chains are PAIRED so each matmul's moving operand is
[128, 256] (two chains' batches side by side), hiding the 107 ns
LDWEIGHTS behind the 256-column stream.  The 4 pairs per core are
interleaved step-by-step so one pair's state update overlaps the other
pairs' matmuls.  Per pair-step: 4 matmuls accumulate a [128, 512] PSUM
tile (one bank), then the update
  - pair 0:      one fused DVE multiply psum(f32) * x -> bf16   (1x rate)
  - pairs 1-3:   ScalarE copies psum -> bf16 SBUF, then DVE multiplies
                 bf16*bf16 at 2x rate
which balances PE / DVE / ScalarE occupancy.

x = exp(emissions) is precomputed host-side in bf16 (memory-bound
problem: this halves HBM traffic and removes the on-chip exp pass).
The gold path score is evaluated on the host.
"""

import numpy as np

B, S, L = 128, 1024, 256
NCORES = 8
NCHAIN = 8              # chains (shards) per core
NPAIR = NCHAIN // 2     # 4 pairs per core
NSH = NCORES * NCHAIN   # 64 shards
BLK = S // NSH          # 16 steps per shard
W = 4                   # warm-up steps per chain
NST = BLK + W           # 20 steps per chain
NPS = NPAIR * NST       # 80 pair-steps per core
TCH = 8                 # pair-steps per DMA chunk
NCHUNK = NPS // TCH     # 10
MU = 6.7
BOS, EOS = 0, 1
DIRECT_PAIRS = (0,)     # pairs whose update is a single fused DVE multiply

_CACHE = {}


def _build_nc():
    import concourse.bacc as bacc
    import concourse.tile as tile
    import concourse.mybir as mybir

    f32 = mybir.dt.float32
    bf16 = mybir.dt.bfloat16
    Act = mybir.ActivationFunctionType

    nc = bacc.Bacc(
        "TRN2", target_bir_lowering=False, debug=False, num_devices=NCORES
    )
    # p-major packed x = exp(emissions), bf16:
    #   [p, pair_step*512 + jc*256 + half*128 + b]
    emis = nc.dram_tensor("emis", [128, NPS * 512], bf16, kind="ExternalInput")
    trans = nc.dram_tensor("trans", [2, 128, 256], f32, kind="ExternalInput")
    teos = nc.dram_tensor("teos", [2, 128, 1], f32, kind="ExternalInput")
    outv = nc.dram_tensor("outv", [1, 2176], f32, kind="ExternalOutput")

    with tile.TileContext(nc) as tc:
        with (
            tc.tile_pool(name="const", bufs=1) as cpool,
            tc.tile_pool(name="stage", bufs=2) as spool,
            tc.tile_pool(name="xchunk", bufs=3) as xpool,
            tc.tile_pool(name="pc", bufs=3) as pcpool,
            tc.tile_pool(name="q0", bufs=3) as qp0,
            tc.tile_pool(name="q1", bufs=3) as qp1,
            tc.tile_pool(name="q2", bufs=3) as qp2,
            tc.tile_pool(name="q3", bufs=3) as qp3,
            tc.tile_pool(name="ps", bufs=4, space="PSUM") as ppool,
            tc.tile_pool(name="psn", bufs=2, space="PSUM") as npool,
            tc.tile_pool(name="outs", bufs=1) as opool,
        ):
            qpools = [qp0, qp1, qp2, qp3]
            negmu = cpool.tile([128, 1], f32, tag="negmu")
            nc.gpsimd.memset(negmu[:], -MU)
            zbias = cpool.tile([128, 1], f32, tag="zbias")
            nc.gpsimd.memset(zbias[:], 0.0)
            zbias1 = cpool.tile([1, 1], f32, tag="zbias1")
            nc.gpsimd.memset(zbias1[:], 0.0)
            # transition weights: expT' = exp(T - MU), bf16, 2 K-chunk tiles
            wT = []
            for ic in range(2):
                st = spool.tile([128, 256], f32, tag="stage", name=f"st{ic}")
                nc.sync.dma_start(st[:], trans[ic])
                w = cpool.tile([128, 256], bf16, tag=f"wT{ic}", name=f"wT{ic}")
                nc.scalar.activation(w[:], st[:], Act.Exp, bias=negmu[:])
                wT.append(w)
            # EOS column weights: exp(T[:, EOS]) (no MU)
            wTe = []
            for ic in range(2):
                st = spool.tile([128, 1], f32, tag="stagee", name=f"ste{ic}")
                nc.sync.dma_start(st[:], teos[ic])
                w = cpool.tile([128, 1], bf16, tag=f"wTe{ic}", name=f"wTe{ic}")
                nc.scalar.activation(w[:], st[:], Act.Exp, bias=zbias[:])
                wTe.append(w)
            ones_col = cpool.tile([128, 1], bf16, tag="ones")
            nc.gpsimd.memset(ones_col[:], 1.0)

            out_sb = opool.tile([1, 2176], f32, tag="outsb")

            # state per pair: [128, 512] bf16; cols = ic*256 + half*128 + b
            qcur = []
            for pi in range(NPAIR):
                q0 = qpools[pi].tile(
                    [128, 512], bf16, tag=f"q{pi}", name=f"qinit{pi}"
                )
                nc.gpsimd.memset(q0[:], 1.0 / L)
                qcur.append(q0)

            for ch in range(NCHUNK):
                xt = xpool.tile([128, TCH * 512], bf16, tag="xt", name=f"xt_{ch}")
                nc.sync.dma_start(
                    xt[:], emis[:, ch * TCH * 512 : (ch + 1) * TCH * 512]
                )

                for s in range(TCH):
                    g = ch * TCH + s       # global pair-step 0..NPS-1
                    pi = g % NPAIR         # which pair
                    step = g // NPAIR      # 0..NST-1 within the pair
                    q = qcur[pi]
                    pts = ppool.tile([128, 512], f32, tag="pt", name=f"pt_{g}")
                    # psum cols = jc*256 + half*128 + b
                    for jc in range(2):
                        for ic in range(2):
                            nc.tensor.matmul(
                                pts[:, jc * 256 : (jc + 1) * 256],
                                wT[ic][:, jc * 128 : (jc + 1) * 128],
                                q[:, ic * 256 : (ic + 1) * 256],
                                start=(ic == 0),
                                stop=(ic == 1),
                            )
                    qn = qpools[pi].tile(
                        [128, 512], bf16, tag=f"q{pi}", name=f"q{pi}_{g}"
                    )
                    xsl = xt[:, s * 512 : (s + 1) * 512]
                    if pi in DIRECT_PAIRS:
                        nc.vector.tensor_mul(qn[:], pts[:], xsl)
                    else:
                        pc = pcpool.tile(
                            [128, 512], bf16, tag="pc", name=f"pc_{g}"
                        )
                        nc.scalar.activation(pc[:], pts[:], Act.Copy)
                        nc.vector.tensor_mul(qn[:], pc[:], xsl)
                    qcur[pi] = qn

                    if step == W - 1 or step == NST - 1:
                        # log-norms per half; chain k = 2*pi + half
                        for half in range(2):
                            k = 2 * pi + half
                            row = 2 * k + (0 if step == W - 1 else 1)
                            nt = npool.tile(
                                [1, 128], f32, tag="nt", name=f"nt_{g}_{half}"
                            )
                            for ic in range(2):
                                nc.tensor.matmul(
                                    nt[:],
                                    ones_col[:],
                                    qn[:, ic * 256 + half * 128 : ic * 256 + half * 128 + 128],
                                    start=(ic == 0),
                                    stop=(ic == 1),
                                )
                            nc.scalar.activation(
                                out_sb[:, row * 128 : (row + 1) * 128],
                                nt[:],
                                Act.Ln,
                                bias=zbias1[:],
                            )
                        if pi == NPAIR - 1 and step == NST - 1:
                            # EOS-weighted log-norm of the core's last chain
                            ntf = npool.tile([1, 128], f32, tag="nt", name="ntf")
                            for ic in range(2):
                                nc.tensor.matmul(
                                    ntf[:],
                                    wTe[ic][:],
                                    qn[:, ic * 256 + 128 : ic * 256 + 256],
                                    start=(ic == 0),
                                    stop=(ic == 1),
                                )
                            nc.scalar.activation(
                                out_sb[:, 2048:2176], ntf[:], Act.Ln,
                                bias=zbias1[:],
                            )

            nc.sync.dma_start(outv[:], out_sb[:])

    nc.compile()
    return nc


def _pack_all(emissions):
    """Pack x = exp(emissions) (B,S,L) into per-core p-major bf16 arrays.

    out[c][p, ((step*NPAIR+pi)*2 + jc)*256 + half*128 + b]
        = exp(emissions[b, t(shard=NCHAIN*c+2*pi+half, step), jc*128+p])
    """
    import ml_dtypes

    x = np.exp(emissions, dtype=np.float32).astype(ml_dtypes.bfloat16)
    el4 = np.ascontiguousarray(x.transpose(2, 1, 0)).reshape(2, 128, S, B)
    tmap = np.empty((NCORES, NPAIR, 2, NST), dtype=np.int64)
    for sh in range(NSH):
        t0 = sh * BLK
        c, r = divmod(sh, NCHAIN)
        pi, half = divmod(r, 2)
        if sh == 0:
            tmap[c, pi, half, : W - 1] = np.arange(W - 1)  # junk steps
            tmap[c, pi, half, W - 1] = 0                   # BOS mask slot
            tmap[c, pi, half, W:] = np.arange(BLK)
        else:
            tmap[c, pi, half] = np.arange(t0 - W, t0 + BLK)
    g = el4[:, :, tmap, :]  # [jc, p, c, pi, half, st, b]
    g = g.transpose(2, 1, 5, 3, 0, 4, 6)  # [c, p, st, pi, jc, half, b]
    ems = np.ascontiguousarray(g.reshape(NCORES, 128, NPS * 512))
    # BOS mask slice (x-domain: onehot) for shard 0: pair-step (W-1)*NPAIR+0
    base = ((W - 1) * NPAIR) * 512
    for jc in range(2):
        ems[0, :, base + jc * 256 : base + jc * 256 + 128] = 0.0
    ems[0, BOS, base : base + 128] = 1.0
    return ems


def kernel(emissions, tags, mask, transitions):
    from concourse.bass_utils import run_bass_kernel_spmd

    emissions = np.asarray(emissions, dtype=np.float32)
    tags_i = np.asarray(tags).astype(np.int64)
    transitions = np.asarray(transitions, dtype=np.float32)

    if "nc" not in _CACHE:
        _CACHE["nc"] = _build_nc()
    nc = _CACHE["nc"]

    trans_in = np.ascontiguousarray(transitions.reshape(2, 128, 256))
    teos_in = np.ascontiguousarray(transitions[:, EOS].reshape(2, 128, 1))

    ems = _pack_all(emissions)
    in_maps = [
        {"emis": ems[c], "trans": trans_in, "teos": teos_in}
        for c in range(NCORES)
    ]

    res = run_bass_kernel_spmd(nc, in_maps, list(range(NCORES)))
    _CACHE["last_res"] = res
    outs = np.stack(
        [np.asarray(r["outv"]).reshape(17, 128) for r in res.results]
    )  # [core, row, b]

    lw = outs[:, 0:16:2, :].astype(np.float64)  # [core, k, b]
    le = outs[:, 1:16:2, :].astype(np.float64)
    fin = outs[-1, 16, :].astype(np.float64)
    logZ = (le - lw).sum(axis=(0, 1)) + S * MU + (fin - le[-1, -1])

    # gold path score on host (tiny: 2*S gathers per sequence)
    em64 = emissions.astype(np.float64)
    T64 = transitions.astype(np.float64)
    e_all = np.take_along_axis(em64, tags_i[..., None], axis=2).squeeze(-1)
    t_all = T64[tags_i[:, :-1], tags_i[:, 1:]]
    scores = (
        T64[BOS, tags_i[:, 0]]
        + e_all[:, 0]
        + (e_all[:, 1:] + t_all).sum(axis=1)
        + T64[tags_i[:, -1], EOS]
    )
    return (logZ - scores).astype(np.float32)


# revision 8
# speedup vs baseline: 2.1595x; 1.0117x over previous
"""CRF NLL kernel for Trainium2 (8 NeuronCores), paired-chain time-sharded
forward algorithm.

Math: NLL[b] = logZ[b] - gold_score[b].

logZ uses the scaled forward algorithm in exp space:
  q_t = (expT'^T q_{t-1}) * x_t,   expT' = exp(T - MU),  x_t = exp(e_t)
so each step is a (256x256) @ (256xB) matmul plus an elementwise multiply.
The constant per-step rescale e^{-MU} keeps magnitudes in fp range.

Sharding: 1024 steps -> 64 shards of 16 steps; each core runs 8 shards
("chains").  Chains warm-start W=4 steps early from a uniform state (the
positive-matrix scan forgets its init fast; validated ~2e-6 rel err with
bf16 x).  Shard 0 instead gets an exact BOS one-hot mask slice.  Scale
invariance gives each shard's contribution from log-norms after warmup
(lw) and at the end (le):
  logZ = sum_c (le_c - lw_c) + S*MU + (fin_63 - le_63)

On-chip layout: chains are PAIRED so each matmul's moving operand is
[128, 256] (two chains' batches side by side), hiding the 107 ns
LDWEIGHTS behind the 256-column stream.  The 4 pairs per core are
interleaved step-by-step so one pair's state update overlaps the other
pairs' matmuls.  Per pair-step: 4 matmuls accumulate a [128, 512] PSUM
tile (one bank), then the update
  - pair 0:      one fused DVE multiply psum(f32) * x -> bf16   (1x rate)
  - pairs 1-3:   ScalarE copies psum -> bf16 SBUF, then DVE multiplies
                 bf16*bf16 at 2x rate
which balances PE / DVE / ScalarE occupancy.

x = exp(emissions) is precomputed host-side in bf16 (memory-bound
problem: this halves HBM traffic and removes the on-chip exp pass).
The gold path score is evaluated on the host.
"""

import numpy as np

B, S, L = 128, 1024, 256
NCORES = 8
NCHAIN = 8              # chains (shards) per core
NPAIR = NCHAIN // 2     # 4 pairs per core
NSH = NCORES * NCHAIN   # 64 shards
BLK = S // NSH          # 16 steps per shard
W = 2                   # warm-up steps per chain
NST = BLK + W           # 18 steps per chain
NPS = NPAIR * NST       # 72 pair-steps per core
TCH = 4                 # pair-steps per DMA chunk
NCHUNK = NPS // TCH     # 18
MU = 6.7
BOS, EOS = 0, 1


def _is_direct(g):
    # ~2/7 of pair-steps take the single fused DVE multiply (1x from PSUM);
    # the rest go ScalarE-copy + DVE 2x, balancing DVE vs ScalarE occupancy.
    return (g % 7) < 2

_CACHE = {}


def _build_nc():
    import concourse.bacc as bacc
    import concourse.tile as tile
    import concourse.mybir as mybir

    f32 = mybir.dt.float32
    bf16 = mybir.dt.bfloat16
    Act = mybir.ActivationFunctionType

    nc = bacc.Bacc(
        "TRN2", target_bir_lowering=False, debug=False, num_devices=NCORES
    )
    # p-major packed x = exp(emissions), bf16:
    #   [p, pair_step*512 + jc*256 + half*128 + b]
    emis = nc.dram_tensor("emis", [128, NPS * 512], bf16, kind="ExternalInput")
    trans = nc.dram_tensor("trans", [2, 128, 256], f32, kind="ExternalInput")
    teos = nc.dram_tensor("teos", [2, 128, 1], f32, kind="ExternalInput")
    outv = nc.dram_tensor("outv", [1, 2176], f32, kind="ExternalOutput")

    with tile.TileContext(nc) as tc:
        with (
            tc.tile_pool(name="const", bufs=1) as cpool,
            tc.tile_pool(name="stage", bufs=2) as spool,
            tc.tile_pool(name="xchunk", bufs=5) as xpool,
            tc.tile_pool(name="pc", bufs=4) as pcpool,
            tc.tile_pool(name="q0", bufs=3) as qp0,
            tc.tile_pool(name="q1", bufs=3) as qp1,
            tc.tile_pool(name="q2", bufs=3) as qp2,
            tc.tile_pool(name="q3", bufs=3) as qp3,
            tc.tile_pool(name="ps", bufs=4, space="PSUM") as ppool,
            tc.tile_pool(name="psn", bufs=2, space="PSUM") as npool,
            tc.tile_pool(name="outs", bufs=1) as opool,
        ):
            qpools = [qp0, qp1, qp2, qp3]
            negmu = cpool.tile([128, 1], f32, tag="negmu")
            nc.gpsimd.memset(negmu[:], -MU)
            zbias = cpool.tile([128, 1], f32, tag="zbias")
            nc.gpsimd.memset(zbias[:], 0.0)
            zbias1 = cpool.tile([1, 1], f32, tag="zbias1")
            nc.gpsimd.memset(zbias1[:], 0.0)
            # transition weights: expT' = exp(T - MU), bf16, 2 K-chunk tiles
            wT = []
            for ic in range(2):
                st = spool.tile([128, 256], f32, tag="stage", name=f"st{ic}")
                nc.sync.dma_start(st[:], trans[ic])
                w = cpool.tile([128, 256], bf16, tag=f"wT{ic}", name=f"wT{ic}")
                nc.scalar.activation(w[:], st[:], Act.Exp, bias=negmu[:])
                wT.append(w)
            # EOS column weights: exp(T[:, EOS]) (no MU)
            wTe = []
            for ic in range(2):
                st = spool.tile([128, 1], f32, tag="stagee", name=f"ste{ic}")
                nc.sync.dma_start(st[:], teos[ic])
                w = cpool.tile([128, 1], bf16, tag=f"wTe{ic}", name=f"wTe{ic}")
                nc.scalar.activation(w[:], st[:], Act.Exp, bias=zbias[:])
                wTe.append(w)
            ones_col = cpool.tile([128, 1], bf16, tag="ones")
            nc.gpsimd.memset(ones_col[:], 1.0)

            out_sb = opool.tile([1, 2176], f32, tag="outsb")

            # state per pair: [128, 512] bf16; cols = ic*256 + half*128 + b
            qcur = []
            for pi in range(NPAIR):
                q0 = qpools[pi].tile(
                    [128, 512], bf16, tag=f"q{pi}", name=f"qinit{pi}"
                )
                nc.gpsimd.memset(q0[:], 1.0 / L)
                qcur.append(q0)

            for ch in range(NCHUNK):
                xt = xpool.tile([128, TCH * 512], bf16, tag="xt", name=f"xt_{ch}")
                nc.sync.dma_start(
                    xt[:], emis[:, ch * TCH * 512 : (ch + 1) * TCH * 512]
                )

                for s in range(TCH):
                    g = ch * TCH + s       # global pair-step 0..NPS-1
                    pi = g % NPAIR         # which pair
                    step = g // NPAIR      # 0..NST-1 within the pair
                    q = qcur[pi]
                    pts = ppool.tile([128, 512], f32, tag="pt", name=f"pt_{g}")
                    # psum cols = jc*256 + half*128 + b
                    for jc in range(2):
                        for ic in range(2):
                            nc.tensor.matmul(
                                pts[:, jc * 256 : (jc + 1) * 256],
                                wT[ic][:, jc * 128 : (jc + 1) * 128],
                                q[:, ic * 256 : (ic + 1) * 256],
                                start=(ic == 0),
                                stop=(ic == 1),
                            )
                    qn = qpools[pi].tile(
                        [128, 512], bf16, tag=f"q{pi}", name=f"q{pi}_{g}"
                    )
                    xsl = xt[:, s * 512 : (s + 1) * 512]
                    if _is_direct(g):
                        nc.vector.tensor_mul(qn[:], pts[:], xsl)
                    else:
                        pc = pcpool.tile(
                            [128, 512], bf16, tag="pc", name=f"pc_{g}"
                        )
                        nc.scalar.activation(pc[:], pts[:], Act.Copy)
                        nc.vector.tensor_mul(qn[:], pc[:], xsl)
                    qcur[pi] = qn

                    if step == W - 1 or step == NST - 1:
                        # log-norms per half; chain k = 2*pi + half
                        for half in range(2):
                            k = 2 * pi + half
                            row = 2 * k + (0 if step == W - 1 else 1)
                            nt = npool.tile(
                                [1, 128], f32, tag="nt", name=f"nt_{g}_{half}"
                            )
                            for ic in range(2):
                                nc.tensor.matmul(
                                    nt[:],
                                    ones_col[:],
                                    qn[:, ic * 256 + half * 128 : ic * 256 + half * 128 + 128],
                                    start=(ic == 0),
                                    stop=(ic == 1),
                                )
                            nc.scalar.activation(
                                out_sb[:, row * 128 : (row + 1) * 128],
                                nt[:],
                                Act.Ln,
                                bias=zbias1[:],
                            )
                        if pi == NPAIR - 1 and step == NST - 1:
                            # EOS-weighted log-norm of the core's last chain
                            ntf = npool.tile([1, 128], f32, tag="nt", name="ntf")
                            for ic in range(2):
                                nc.tensor.matmul(
                                    ntf[:],
                                    wTe[ic][:],
                                    qn[:, ic * 256 + 128 : ic * 256 + 256],
                                    start=(ic == 0),
                                    stop=(ic == 1),
                                )
                            nc.scalar.activation(
                                out_sb[:, 2048:2176], ntf[:], Act.Ln,
                                bias=zbias1[:],
                            )

            nc.sync.dma_start(outv[:], out_sb[:])

    nc.compile()
    return nc


def _pack_all(emissions):
    """Pack x = exp(emissions) (B,S,L) into per-core p-major bf16 arrays.

    out[c][p, ((step*NPAIR+pi)*2 + jc)*256 + half*128 + b]
        = exp(emissions[b, t(shard=NCHAIN*c+2*pi+half, step), jc*128+p])
    """
    import ml_dtypes

    x = np.exp(emissions, dtype=np.float32).astype(ml_dtypes.bfloat16)
    el4 = np.ascontiguousarray(x.transpose(2, 1, 0)).reshape(2, 128, S, B)
    tmap = np.empty((NCORES, NPAIR, 2, NST), dtype=np.int64)
    for sh in range(NSH):
        t0 = sh * BLK
        c, r = divmod(sh, NCHAIN)
        pi, half = divmod(r, 2)
        if sh == 0:
            tmap[c, pi, half, : W - 1] = np.arange(W - 1)  # junk steps
            tmap[c, pi, half, W - 1] = 0                   # BOS mask slot
            tmap[c, pi, half, W:] = np.arange(BLK)
        else:
            tmap[c, pi, half] = np.arange(t0 - W, t0 + BLK)
    g = el4[:, :, tmap, :]  # [jc, p, c, pi, half, st, b]
    g = g.transpose(2, 1, 5, 3, 0, 4, 6)  # [c, p, st, pi, jc, half, b]
    ems = np.ascontiguousarray(g.reshape(NCORES, 128, NPS * 512))
    # BOS mask slice (x-domain: onehot) for shard 0: pair-step (W-1)*NPAIR+0
    base = ((W - 1) * NPAIR) * 512
    for jc in range(2):
        ems[0, :, base + jc * 256 : base + jc * 256 + 128] = 0.0
    ems[0, BOS, base : base + 128] = 1.0
    return ems


def kernel(emissions, tags, mask, transitions):
    from concourse.bass_utils import run_bass_kernel_spmd

    emissions = np.asarray(emissions, dtype=np.float32)
    tags_i = np.asarray(tags).astype(np.int64)
    transitions = np.asarray(transitions, dtype=np.float32)

    if "nc" not in _CACHE:
        _CACHE["nc"] = _build_nc()
    nc = _CACHE["nc"]

    trans_in = np.ascontiguousarray(transitions.reshape(2, 128, 256))
    teos_in = np.ascontiguousarray(transitions[:, EOS].reshape(2, 128, 1))

    ems = _pack_all(emissions)
    in_maps = [
        {"emis": ems[c], "trans": trans_in, "teos": teos_in}
        for c in range(NCORES)
    ]

    res = run_bass_kernel_spmd(nc, in_maps, list(range(NCORES)))
    _CACHE["last_res"] = res
    outs = np.stack(
        [np.asarray(r["outv"]).reshape(17, 128) for r in res.results]
    )  # [core, row, b]

    lw = outs[:, 0:16:2, :].astype(np.float64)  # [core, k, b]
    le = outs[:, 1:16:2, :].astype(np.float64)
    fin = outs[-1, 16, :].astype(np.float64)
    logZ = (le - lw).sum(axis=(0, 1)) + S * MU + (fin - le[-1, -1])

    # gold path score on host (tiny: 2*S gathers per sequence)
    em64 = emissions.astype(np.float64)
    T64 = transitions.astype(np.float64)
    e_all = np.take_along_axis(em64, tags_i[..., None], axis=2).squeeze(-1)
    t_all = T64[tags_i[:, :-1], tags_i[:, 1:]]
    scores = (
        T64[BOS, tags_i[:, 0]]
        + e_all[:, 0]
        + (e_all[:, 1:] + t_all).sum(axis=1)
        + T64[tags_i[:, -1], EOS]
    )
    return (logZ - scores).astype(np.float32)


# revision 10
# speedup vs baseline: 2.8644x; 1.3264x over previous
"""CRF NLL kernel for Trainium2 (8 NeuronCores), paired-chain time-sharded
forward algorithm.

Math: NLL[b] = logZ[b] - gold_score[b].

logZ uses the scaled forward algorithm in exp space:
  q_t = (expT'^T q_{t-1}) * x_t,   expT' = exp(T - MU),  x_t = exp(e_t)
so each step is a (256x256) @ (256xB) matmul plus an elementwise multiply.
The constant per-step rescale e^{-MU} keeps magnitudes in fp range.

Sharding: 1024 steps -> 128 shards of 8 steps; each core runs 16 shards
("chains"), all started directly from a uniform state with NO warm-up:
the positive-matrix scan contracts so hard that the block-telescoped
  logZ = sum_c le_c + (S-1)*MU + (fin_last - le_last)
(le_c = log-norm of chain c's end state; the uniform start has log-norm
exactly 0) is accurate to ~6e-5 relative (validated in f64+bf16-x).
Shard 0's exact BOS initial condition is folded into its first x slice
on the host: x'_0[l,b] = exp(e_0[l,b]) * exp(T[BOS,l]) / mean_i expT'[i,l],
which makes chain 0 exact (its step 0 then carries no e^{-MU}, hence the
(S-1) factor).

On-chip layout: chains are PAIRED so each matmul's moving operand is
[128, 256] (two chains' batches side by side), hiding the 107 ns
LDWEIGHTS behind the 256-column stream.  The 8 pairs per core are
interleaved step-by-step, giving the round-robin enough slack to hide
the PE->DVE/ScalarE->PE dependency latency of each pair.  Per pair-step:
4 matmuls accumulate a [128, 512] PSUM tile (one bank), then the update
  - ~2/7 of steps: one fused DVE multiply psum(f32) * x -> bf16 (1x rate)
  - the rest:      ScalarE copies psum -> bf16 SBUF, then DVE multiplies
                   bf16*bf16 at 2x rate
which balances PE / DVE / ScalarE occupancy.

x = exp(emissions) and the bf16 weights are precomputed host-side
(memory-bound problem: bf16 x halves HBM traffic; ready-made weights
remove the on-chip weight-prep chain from the critical startup path).
The gold path score is evaluated on the host.
"""

import numpy as np

B, S, L = 128, 1024, 256
NCORES = 8
NCHAIN = 16             # chains (shards) per core
NPAIR = NCHAIN // 2     # 8 pairs per core
NSH = NCORES * NCHAIN   # 128 shards
BLK = S // NSH          # 8 steps per shard
NST = BLK               # steps per chain (no warm-up)
NPS = NPAIR * NST       # 64 pair-steps per core
TCH = 4                 # pair-steps per DMA chunk
NCHUNK = NPS // TCH     # 16
MU = 6.7
BOS, EOS = 0, 1

_CACHE = {}


def _is_direct(g):
    # ~2/7 of pair-steps take the single fused DVE multiply (1x from PSUM);
    # the rest go ScalarE-copy + DVE 2x, balancing DVE vs ScalarE occupancy.
    return (g % 7) < 2


def _build_nc():
    import concourse.bacc as bacc
    import concourse.tile as tile
    import concourse.mybir as mybir

    f32 = mybir.dt.float32
    bf16 = mybir.dt.bfloat16
    Act = mybir.ActivationFunctionType

    nc = bacc.Bacc(
        "TRN2", target_bir_lowering=False, debug=False, num_devices=NCORES
    )
    # p-major packed x = exp(emissions), bf16:
    #   [p, pair_step*512 + jc*256 + half*128 + b]
    emis = nc.dram_tensor("emis", [128, NPS * 512], bf16, kind="ExternalInput")
    # precomputed weights: wt[ic][p, j] = exp(T[ic*128+p, j] - MU)
    wt_in = nc.dram_tensor("wt", [2, 128, 256], bf16, kind="ExternalInput")
    # wte[ic][p, 0] = exp(T[ic*128+p, EOS])
    wte_in = nc.dram_tensor("wte", [2, 128, 1], bf16, kind="ExternalInput")
    outv = nc.dram_tensor("outv", [1, 2176], f32, kind="ExternalOutput")

    with tile.TileContext(nc) as tc:
        with (
            tc.tile_pool(name="const", bufs=1) as cpool,
            tc.tile_pool(name="xchunk", bufs=5) as xpool,
            tc.tile_pool(name="pc", bufs=4) as pcpool,
            tc.tile_pool(name="qs", bufs=3) as qpool,
            tc.tile_pool(name="ps", bufs=4, space="PSUM") as ppool,
            tc.tile_pool(name="psn", bufs=2, space="PSUM") as npool,
            tc.tile_pool(name="outs", bufs=1) as opool,
        ):
            wT = []
            for ic in range(2):
                w = cpool.tile([128, 256], bf16, tag=f"wT{ic}", name=f"wT{ic}")
                nc.sync.dma_start(w[:], wt_in[ic])
                wT.append(w)
            wTe = []
            for ic in range(2):
                w = cpool.tile([128, 1], bf16, tag=f"wTe{ic}", name=f"wTe{ic}")
                nc.sync.dma_start(w[:], wte_in[ic])
                wTe.append(w)
            ones_col = cpool.tile([128, 1], bf16, tag="ones")
            nc.vector.memset(ones_col[:], 1.0)
            zbias1 = cpool.tile([1, 1], f32, tag="zbias1")
            nc.vector.memset(zbias1[:], 0.0)

            out_sb = opool.tile([1, 2176], f32, tag="outsb")

            # state per pair: [128, 512] bf16; cols = ic*256 + half*128 + b
            qcur = []
            for pi in range(NPAIR):
                q0 = qpool.tile([128, 512], bf16, tag=f"q{pi}", name=f"qi{pi}")
                nc.vector.memset(q0[:], 1.0 / L)
                qcur.append(q0)

            for ch in range(NCHUNK):
                xt = xpool.tile([128, TCH * 512], bf16, tag="xt", name=f"xt_{ch}")
                nc.sync.dma_start(
                    xt[:], emis[:, ch * TCH * 512 : (ch + 1) * TCH * 512]
                )

                for s in range(TCH):
                    g = ch * TCH + s       # global pair-step 0..NPS-1
                    pi = g % NPAIR         # which pair
                    step = g // NPAIR      # 0..NST-1 within the pair
                    q = qcur[pi]
                    pts = ppool.tile([128, 512], f32, tag="pt", name=f"pt_{g}")
                    # psum cols = jc*256 + half*128 + b
                    for jc in range(2):
                        for ic in range(2):
                            nc.tensor.matmul(
                                pts[:, jc * 256 : (jc + 1) * 256],
                                wT[ic][:, jc * 128 : (jc + 1) * 128],
                                q[:, ic * 256 : (ic + 1) * 256],
                                start=(ic == 0),
                                stop=(ic == 1),
                            )
                    qn = qpool.tile(
                        [128, 512], bf16, tag=f"q{pi}", name=f"q{pi}_{g}"
                    )
                    xsl = xt[:, s * 512 : (s + 1) * 512]
                    if _is_direct(g):
                        nc.vector.tensor_mul(qn[:], pts[:], xsl)
                    else:
                        pc = pcpool.tile(
                            [128, 512], bf16, tag="pc", name=f"pc_{g}"
                        )
                        nc.scalar.activation(pc[:], pts[:], Act.Copy)
                        nc.vector.tensor_mul(qn[:], pc[:], xsl)
                    qcur[pi] = qn

                    if step == NST - 1:
                        # end-of-chain log-norms; chain k = 2*pi + half
                        for half in range(2):
                            k = 2 * pi + half
                            nt = npool.tile(
                                [1, 128], f32, tag="nt", name=f"nt_{g}_{half}"
                            )
                            for ic in range(2):
                                nc.tensor.matmul(
                                    nt[:],
                                    ones_col[:],
                                    qn[:, ic * 256 + half * 128 : ic * 256 + half * 128 + 128],
                                    start=(ic == 0),
                                    stop=(ic == 1),
                                )
                            nc.scalar.activation(
                                out_sb[:, k * 128 : (k + 1) * 128],
                                nt[:],
                                Act.Ln,
                                bias=zbias1[:],
                            )
                        if pi == NPAIR - 1:
                            # EOS-weighted log-norm of the core's last chain
                            ntf = npool.tile([1, 128], f32, tag="nt", name="ntf")
                            for ic in range(2):
                                nc.tensor.matmul(
                                    ntf[:],
                                    wTe[ic][:],
                                    qn[:, ic * 256 + 128 : ic * 256 + 256],
                                    start=(ic == 0),
                                    stop=(ic == 1),
                                )
                            nc.scalar.activation(
                                out_sb[:, 2048:2176], ntf[:], Act.Ln,
                                bias=zbias1[:],
                            )

            nc.sync.dma_start(outv[:], out_sb[:])

    nc.compile()
    return nc


def _pack_all(emissions, transitions):
    """Pack x = exp(emissions) (B,S,L) into per-core p-major bf16 arrays.

    out[c][p, ((step*NPAIR+pi)*2 + jc)*256 + half*128 + b]
        = exp(emissions[b, t(shard=NCHAIN*c+2*pi+half, step), jc*128+p])

    Shard 0's first slice gets the BOS fold:
        x'_0[l,b] = exp(e_0[l,b]) * exp(T[BOS,l]) / mean_i exp(T[i,l]-MU)
    """
    import ml_dtypes

    x = np.exp(emissions, dtype=np.float32).astype(ml_dtypes.bfloat16)
    el4 = np.ascontiguousarray(x.transpose(2, 1, 0)).reshape(2, 128, S, B)
    tmap = np.empty((NCORES, NPAIR, 2, NST), dtype=np.int64)
    for sh in range(NSH):
        t0 = sh * BLK
        c, r = divmod(sh, NCHAIN)
        pi, half = divmod(r, 2)
        tmap[c, pi, half] = np.arange(t0, t0 + BLK)
    g = el4[:, :, tmap, :]  # [jc, p, c, pi, half, st, b]
    g = g.transpose(2, 1, 5, 3, 0, 4, 6)  # [c, p, st, pi, jc, half, b]
    ems = np.ascontiguousarray(g.reshape(NCORES, 128, NPS * 512))
    # BOS fold for shard 0 (core 0, pair 0, half 0, step 0 -> pair-step 0)
    T64 = transitions.astype(np.float64)
    cfac = np.exp(T64[BOS, :]) / np.exp(T64 - MU).mean(axis=0)  # (L,)
    x0 = (
        np.exp(emissions[:, 0, :].astype(np.float64)) * cfac[None, :]
    ).astype(np.float32).astype(ml_dtypes.bfloat16)  # [b, l]
    x0 = x0.T.reshape(2, 128, B)  # [jc, p, b]
    for jc in range(2):
        ems[0, :, jc * 256 : jc * 256 + 128] = x0[jc]
    return ems


def kernel(emissions, tags, mask, transitions):
    import ml_dtypes
    from concourse.bass_utils import run_bass_kernel_spmd

    emissions = np.asarray(emissions, dtype=np.float32)
    tags_i = np.asarray(tags).astype(np.int64)
    transitions = np.asarray(transitions, dtype=np.float32)

    if "nc" not in _CACHE:
        _CACHE["nc"] = _build_nc()
    nc = _CACHE["nc"]

    wt_in = np.ascontiguousarray(
        np.exp(transitions - MU).astype(ml_dtypes.bfloat16).reshape(2, 128, 256)
    )
    wte_in = np.ascontiguousarray(
        np.exp(transitions[:, EOS]).astype(ml_dtypes.bfloat16).reshape(2, 128, 1)
    )

    ems = _pack_all(emissions, transitions)
    in_maps = [
        {"emis": ems[c], "wt": wt_in, "wte": wte_in} for c in range(NCORES)
    ]

    res = run_bass_kernel_spmd(nc, in_maps, list(range(NCORES)))
    _CACHE["last_res"] = res
    outs = np.stack(
        [np.asarray(r["outv"]).reshape(17, 128) for r in res.results]
    )  # [core, row, b]

    le = outs[:, 0:16, :].astype(np.float64)  # [core, k, b]
    fin = outs[-1, 16, :].astype(np.float64)
    logZ = le.sum(axis=(0, 1)) + (S - 1) * MU + (fin - le[-1, -1])

    # gold path score on host (tiny: 2*S gathers per sequence)
    em64 = emissions.astype(np.float64)
    T64 = transitions.astype(np.float64)
    e_all = np.take_along_axis(em64, tags_i[..., None], axis=2).squeeze(-1)
    t_all = T64[tags_i[:, :-1], tags_i[:, 1:]]
    scores = (
        T64[BOS, tags_i[:, 0]]
        + e_all[:, 0]
        + (e_all[:, 1:] + t_all).sum(axis=1)
        + T64[tags_i[:, -1], EOS]
    )
    return (logZ - scores).astype(np.float32)


# revision 11
# speedup vs baseline: 2.9671x; 1.0359x over previous
"""CRF NLL kernel for Trainium2 (8 NeuronCores), quad-chain time-sharded
forward algorithm.

Math: NLL[b] = logZ[b] - gold_score[b].

logZ uses the scaled forward algorithm in exp space:
  q_t = (expT'^T q_{t-1}) * x_t,   expT' = exp(T - MU),  x_t = exp(e_t)
so each step is a (256x256) @ (256xB) matmul plus an elementwise multiply.
The constant per-step rescale e^{-MU} keeps magnitudes in fp range.

Sharding: 1024 steps -> 128 shards of 8 steps; each core runs 16 shards
("chains"), all started directly from a uniform state with NO warm-up:
the positive-matrix scan contracts so hard that the block-telescoped
  logZ = sum_c le_c + (S-1)*MU + (fin_last - le_last)
(le_c = log-norm of chain c's end state; the uniform start has log-norm
exactly 0) is accurate to ~6e-5 relative (validated in f64+bf16-x).
Shard 0's exact BOS initial condition is folded into its first x slice
on the host, making chain 0 exact (its step 0 then carries no e^{-MU},
hence the (S-1) factor).

On-chip layout: chains are grouped in QUADS so each matmul's moving
operand is [128, 512] (four chains' batches side by side), hiding the
LDWEIGHTS behind the 512-column stream.  The 4 quads per core are
interleaved step-by-step, giving the round-robin enough slack to hide
each quad's PE -> DVE/ScalarE -> PE dependency latency.  Per quad-step:
4 matmuls accumulate a [128, 1024] PSUM tile (2 banks, one matmul
output region per bank), then the state update
  - ~30% of steps: one fused DVE multiply psum(f32) * x -> bf16 (1x)
  - the rest:      ScalarE copies psum -> bf16 SBUF, then DVE multiplies
                   bf16*bf16 at 2x rate
which balances PE / DVE / ScalarE occupancy.  Final quad states are
DMA'd to HBM; the log-norms (and the EOS-weighted fin) are computed on
the host in f64, removing the norm-matmul/Ln tail from the kernel.

x = exp(emissions) and the bf16 weights are precomputed host-side.
The gold path score is evaluated on the host.
"""

import numpy as np

B, S, L = 128, 1024, 256
NCORES = 8
NCHAIN = 16             # chains (shards) per core
NQUAD = NCHAIN // 4     # 4 quads per core
NSH = NCORES * NCHAIN   # 128 shards
BLK = S // NSH          # 8 steps per shard
NST = BLK               # steps per chain (no warm-up)
NQS = NQUAD * NST       # 32 quad-steps per core
TCH = 2                 # quad-steps per DMA chunk
NCHUNK = NQS // TCH     # 16
MU = 6.7
BOS, EOS = 0, 1

_CACHE = {}


def _is_direct(gq):
    # ~30% of quad-steps take the single fused DVE multiply (1x from PSUM);
    # the rest go ScalarE-copy + DVE 2x, balancing DVE vs ScalarE occupancy.
    return (gq % 10) < 3


def _build_nc():
    import concourse.bacc as bacc
    import concourse.tile as tile
    import concourse.mybir as mybir

    f32 = mybir.dt.float32
    bf16 = mybir.dt.bfloat16
    Act = mybir.ActivationFunctionType

    nc = bacc.Bacc(
        "TRN2", target_bir_lowering=False, debug=False, num_devices=NCORES
    )
    # p-major packed x = exp(emissions), bf16:
    #   [p, quad_step*1024 + jc*512 + u*256 + half*128 + b]
    emis = nc.dram_tensor("emis", [128, NQS * 1024], bf16, kind="ExternalInput")
    # precomputed weights: wt[ic][p, j] = exp(T[ic*128+p, j] - MU)
    wt_in = nc.dram_tensor("wt", [2, 128, 256], bf16, kind="ExternalInput")
    # final states of the 4 quads, unpacked host-side for norms/fin
    outq = nc.dram_tensor("outq", [NQUAD, 128, 1024], bf16, kind="ExternalOutput")

    with tile.TileContext(nc) as tc:
        with (
            tc.tile_pool(name="const", bufs=1) as cpool,
            tc.tile_pool(name="xchunk", bufs=5) as xpool,
            tc.tile_pool(name="pc", bufs=4) as pcpool,
            tc.tile_pool(name="qs", bufs=3) as qpool,
            tc.tile_pool(name="ps", bufs=3, space="PSUM") as ppool,
        ):
            wT = []
            for ic in range(2):
                w = cpool.tile([128, 256], bf16, tag=f"wT{ic}", name=f"wT{ic}")
                nc.sync.dma_start(w[:], wt_in[ic])
                wT.append(w)

            # state per quad: [128, 1024] bf16; cols = ic*512 + u*256 + half*128 + b
            # single big init tile memset once (gpsimd: off the critical engines)
            qinit = cpool.tile([128, NQUAD * 1024], bf16, tag="qinit")
            nc.gpsimd.memset(qinit[:], 1.0 / L)
            qcur = [qinit[:, qi * 1024 : (qi + 1) * 1024] for qi in range(NQUAD)]

            for ch in range(NCHUNK):
                xt = xpool.tile([128, TCH * 1024], bf16, tag="xt", name=f"xt_{ch}")
                nc.sync.dma_start(
                    xt[:], emis[:, ch * TCH * 1024 : (ch + 1) * TCH * 1024]
                )

                for s in range(TCH):
                    gq = ch * TCH + s      # global quad-step 0..NQS-1
                    qi = gq % NQUAD        # which quad
                    step = gq // NQUAD     # 0..NST-1 within the quad
                    q = qcur[qi]
                    pts = ppool.tile([128, 1024], f32, tag="pt", name=f"pt_{gq}")
                    # psum cols = jc*512 + u*256 + half*128 + b (bank per jc)
                    for jc in range(2):
                        for ic in range(2):
                            nc.tensor.matmul(
                                pts[:, jc * 512 : (jc + 1) * 512],
                                wT[ic][:, jc * 128 : (jc + 1) * 128],
                                q[:, ic * 512 : (ic + 1) * 512],
                                start=(ic == 0),
                                stop=(ic == 1),
                            )
                    qn = qpool.tile(
                        [128, 1024], bf16, tag=f"q{qi}", name=f"q{qi}_{gq}"
                    )
                    xsl = xt[:, s * 1024 : (s + 1) * 1024]
                    if _is_direct(gq):
                        nc.vector.tensor_mul(qn[:], pts[:], xsl)
                    else:
                        pc = pcpool.tile(
                            [128, 1024], bf16, tag="pc", name=f"pc_{gq}"
                        )
                        nc.scalar.activation(pc[:], pts[:], Act.Copy)
                        nc.vector.tensor_mul(qn[:], pc[:], xsl)
                    qcur[qi] = qn[:]

                    if step == NST - 1:
                        nc.sync.dma_start(outq[qi], qn[:])

    nc.compile()
    return nc


def _pack_all(emissions, transitions):
    """Pack x = exp(emissions) (B,S,L) into per-core p-major bf16 arrays.

    out[c][p, ((st*NQUAD+qi)*2 + jc)*512 + u*256 + half*128 + b]
        = exp(emissions[b, t(shard, st), jc*128+p]),
    shard = 16*c + 4*qi + 2*u + half.

    Shard 0's first slice gets the BOS fold:
        x'_0[l,b] = exp(e_0[l,b]) * exp(T[BOS,l]) / mean_i exp(T[i,l]-MU)
    """
    import ml_dtypes

    x = np.exp(emissions, dtype=np.float32).astype(ml_dtypes.bfloat16)
    el4 = np.ascontiguousarray(x.transpose(2, 1, 0)).reshape(2, 128, S, B)
    tmap = np.empty((NCORES, NQUAD, 2, 2, NST), dtype=np.int64)
    for sh in range(NSH):
        t0 = sh * BLK
        c, r = divmod(sh, NCHAIN)
        qi, r2 = divmod(r, 4)
        u, half = divmod(r2, 2)
        tmap[c, qi, u, half] = np.arange(t0, t0 + BLK)
    g = el4[:, :, tmap, :]  # [jc, p, c, qi, u, half, st, b]
    g = g.transpose(2, 1, 6, 3, 0, 4, 5, 7)  # [c, p, st, qi, jc, u, half, b]
    ems = np.ascontiguousarray(g.reshape(NCORES, 128, NQS * 1024))
    # BOS fold for shard 0 (core 0, quad 0, u 0, half 0, step 0 -> quad-step 0)
    T64 = transitions.astype(np.float64)
    cfac = np.exp(T64[BOS, :]) / np.exp(T64 - MU).mean(axis=0)  # (L,)
    x0 = (
        np.exp(emissions[:, 0, :].astype(np.float64)) * cfac[None, :]
    ).astype(np.float32).astype(ml_dtypes.bfloat16)  # [b, l]
    x0 = x0.T.reshape(2, 128, B)  # [jc, p, b]
    for jc in range(2):
        ems[0, :, jc * 512 : jc * 512 + 128] = x0[jc]
    return ems


def kernel(emissions, tags, mask, transitions):
    import ml_dtypes
    from concourse.bass_utils import run_bass_kernel_spmd

    emissions = np.asarray(emissions, dtype=np.float32)
    tags_i = np.asarray(tags).astype(np.int64)
    transitions = np.asarray(transitions, dtype=np.float32)

    if "nc" not in _CACHE:
        _CACHE["nc"] = _build_nc()
    nc = _CACHE["nc"]

    wt_in = np.ascontiguousarray(
        np.exp(transitions - MU).astype(ml_dtypes.bfloat16).reshape(2, 128, 256)
    )

    ems = _pack_all(emissions, transitions)
    in_maps = [{"emis": ems[c], "wt": wt_in} for c in range(NCORES)]

    res = run_bass_kernel_spmd(nc, in_maps, list(range(NCORES)))
    _CACHE["last_res"] = res

    # unpack final states: outq[qi][p, ic*512 + u*256 + half*128 + b]
    # -> q_end[chain k = 4*qi+2*u+half][label ic*128+p, b]
    T64 = transitions.astype(np.float64)
    le = np.empty((NCORES, NCHAIN, B))
    fin = None
    for c in range(NCORES):
        oq = np.asarray(res.results[c]["outq"]).astype(np.float64)
        # [qi, p, ic*512 + u*256 + half*128 + b]
        oq = oq.reshape(NQUAD, 128, 2, 2, 2, 128)  # [qi, p, ic, u, half, b]
        for qi in range(NQUAD):
            for u in range(2):
                for half in range(2):
                    k = 4 * qi + 2 * u + half
                    qend = oq[qi, :, :, u, half, :]  # [p, ic, b]
                    le[c, k] = np.log(qend.sum(axis=(0, 1)))
                    if c == NCORES - 1 and k == NCHAIN - 1:
                        wte = np.exp(T64[:, EOS]).reshape(2, 128).T  # [p, ic]
                        fin = np.log(
                            (qend * wte[:, :, None]).sum(axis=(0, 1))
                        )
    logZ = le.sum(axis=(0, 1)) + (S - 1) * MU + (fin - le[-1, -1])

    # gold path score on host (tiny: 2*S gathers per sequence)
    em64 = emissions.astype(np.float64)
    e_all = np.take_along_axis(em64, tags_i[..., None], axis=2).squeeze(-1)
    t_all = T64[tags_i[:, :-1], tags_i[:, 1:]]
    scores = (
        T64[BOS, tags_i[:, 0]]
        + e_all[:, 0]
        + (e_all[:, 1:] + t_all).sum(axis=1)
        + T64[tags_i[:, -1], EOS]
    )
    return (logZ - scores).astype(np.float32)
